# revision 2
# baseline (speedup 1.0000x reference)
"""Trainium2 Bass kernel for nn_BlockCrossAttn (block-diagonal attention, E=H=1).

Math per (block b, batch n) pair (256-long vectors q', k', v of the block):
    q' = wq*Q + bq ; k' = wk*K + bk
    soft[q,k] = softmax_k(q'[q] * k'[k])
    out[q] = wvo * (sum_k soft[q,k] * V[k]) + (bvo + bo)
where wvo = wo*wv, bvo = wo*bv (the V/out affine folds into the epilogue
because softmax weights sum to 1).  No max-subtraction: |scores| <= ~27
worst case, exp is safe in fp32.

Sharding: 128 blocks of 256 rows; 16 blocks per core across 8 cores
(fully independent, no collectives).

Per-core device pipeline (512 pairs):
  - PE outer products (contraction dim 1) in fp16 (full bf16-rate, 11-bit
    mantissa) build S^T[k, q] in PSUM, 3 pairs per 3-bank group, double
    buffered.
  - ScalarE exp over [128, 1536] PSUM spans -> E (fp16) in SBUF.
  - PE reduction matmuls: lhsT = [ones, v_hi, v_lo] fp16 3-column AP, rhs =
    E streams -> PSUM [3, 256] = (denom, numer_hi, numer_lo) rows per pair;
    every matmul start=True/stop=True (no PSUM accumulation -> no whole-bank
    has_written hazards); 4 pairs per result bank.
  - VectorE flushes banks to SBUF; a DRAM scratch bounce re-lays 32 pairs
    into a dense [32, 1536] tile (one writer); VectorE adds the two ktile
    partials, reciprocal_approx_fast + multiply + affine epilogue;
    one contiguous DMA per block to the n-major output.

Weight scalars are baked into the module as immediates (compiled per
weight set, cached) to avoid TensorScalarPtr sync-wait limits.
"""

from contextlib import ExitStack

import numpy as np

import concourse.bacc as bacc
import concourse.bass as bass
import concourse.tile as tile
from concourse import mybir
from concourse.bass_utils import run_bass_kernel_spmd

FP = mybir.dt.float32
AF = mybir.ActivationFunctionType
ALU = mybir.AluOpType

L = 32768          # sequence length
N = 32             # batch
BS = 256           # block size
NB = L // BS       # 128 blocks
NCORES = 8
BPC = NB // NCORES  # 16 blocks per core
LS = BPC * BS       # 4096 rows per core shard

GROUP = 3           # pairs per exp staging group (3 PSUM banks)
PAIRS = BPC * N     # 512 pairs per core
F16 = mybir.dt.float16
QKDT = F16          # score matmul operand dtype (full-rate, 11-bit mantissa)
EDT = F16           # E dtype for the reductions


def build_kernel_module(sc, reps: int = 1) -> bass.Bass:
    """sc: dict of python-float weight scalars baked as immediates.

    reps > 1 wraps the whole body in a device-side For_i loop — used only
    for wall-clock benchmarking (amplifies device time over dispatch noise).
    """
    nc = bacc.Bacc("TRN2", target_bir_lowering=False, debug=False, num_devices=NCORES)
    # qkt[4n+c, :] = [qT[n, 1024c:1024(c+1)] | kT[n, 1024c:1024(c+1)]]
    qkt = nc.declare_dram_parameter("qkt", [128, 2048], FP, isOutput=False)
    v = nc.declare_dram_parameter("v", [LS, N], FP, isOutput=False)
    out_t = nc.declare_dram_parameter("out_t", [N, LS], FP, isOutput=True)

    with tile.TileContext(nc) as tc:
        with ExitStack() as ctx:
            if reps == 1:
                _emit(ctx, tc, qkt, v, out_t, sc)
            else:
                with tc.For_i(0, reps, 1):
                    _emit(ctx, tc, qkt, v, out_t, sc)
    nc.compile()
    return nc


def _emit(ctx, tc, qkt, v, out_t, sc):
    nc = tc.nc

    rows = ctx.enter_context(tc.tile_pool(name="rows", bufs=1))
    stage = ctx.enter_context(tc.tile_pool(name="stage", bufs=2))
    vpool = ctx.enter_context(tc.tile_pool(name="vpool", bufs=1))
    epool = ctx.enter_context(tc.tile_pool(name="epool", bufs=3))
    dpool = ctx.enter_context(tc.tile_pool(name="dpool", bufs=2))
    ps_stage = ctx.enter_context(tc.tile_pool(name="ps_stage", bufs=2, space="PSUM"))
    ps_res = ctx.enter_context(tc.tile_pool(name="ps_res", bufs=2, space="PSUM"))
    drs = ctx.enter_context(tc.tile_pool(name="drs", bufs=2, space="DRAM"))

    # --- prep ------------------------------------------------------------------
    qk32 = rows.tile([128, 2048], FP, name="qk32", tag="qk32")
    nc.sync.dma_start(out=qk32[:], in_=qkt[:])
    # affine + cast to fp16 for the score matmuls
    qk4 = rows.tile([128, 2048], QKDT, name="qk4", tag="qk4")
    nc.vector.tensor_scalar(
        out=qk4[:, 0:1024], in0=qk32[:, 0:1024],
        scalar1=sc["wq"], scalar2=sc["bq"], op0=ALU.mult, op1=ALU.add,
    )
    nc.vector.tensor_scalar(
        out=qk4[:, 1024:2048], in0=qk32[:, 1024:2048],
        scalar1=sc["wk"], scalar2=sc["bk"], op0=ALU.mult, op1=ALU.add,
    )

    # [ones, v_hi, v_lo] tiles: col 0 = 1.0 (memset once); cols 1,2 = fp16
    # hi/lo split of raw V of the block, [t, n] order.  Two fixed tiles used
    # alternately per block.
    vcombs = []
    for name in ("vcA", "vcB"):
        vc = vpool.tile([128, 2, N, 3], EDT, name=name, tag=name)
        nc.vector.memset(vc[:], 1.0)
        vcombs.append(vc)

    def load_vcomb(b):
        # DMA raw V, then split into fp16 hi+lo columns (exact to ~2^-22).
        vc = vcombs[b % 2]
        vch = vpool.tile([128, 2, N], FP, name="vch", tag="vch", bufs=2)
        hi32 = vpool.tile([128, 2, N], FP, name="hi32", tag="hi32", bufs=2)
        nc.sync.dma_start(
            out=vch[:],
            in_=v[b * BS:(b + 1) * BS, :].rearrange("(t p) n -> p t n", p=128),
        )
        vc4 = vc[:]
        nc.vector.tensor_copy(vc4[:, :, :, 1], vch[:])
        nc.vector.tensor_copy(hi32[:], vc4[:, :, :, 1])
        nc.vector.tensor_sub(vc4[:, :, :, 2], vch[:], hi32[:])
        return vc

    # --- per-half-block q/k row staging (to partition 0) -----------------------
    def stage_rows(b, h):
        # row n (16h <= n < 16h+16): q at [0, (2(n-16h))*256:...],
        #                            k at [0, (2(n-16h)+1)*256:...]
        qks = stage.tile([1, 16 * 2 * BS], QKDT, name="qks", tag="qks")
        qv = qk4[:].rearrange("(n c) (g f) -> n c g f", c=4, g=2)
        cb, cc = b // 4, (b % 4) * BS
        nc.sync.dma_start(out=qks[:], in_=qv[16 * h:16 * (h + 1), cb, :, cc:cc + BS])
        return qks

    # --- main loop --------------------------------------------------------------
    vcur = [None]
    res_state = {"tile": None, "count": 0, "nflush": 0, "rs": None, "first_g": 0}

    def emit_reduces(pend):
        e, members = pend
        for (s, b, n, vc) in members:
            g = b * N + n
            r = res_state["count"]
            if r == 0:
                res_state["tile"] = ps_res.tile([128, 512], FP, name="res", tag="res")
                if res_state["nflush"] == 0:
                    res_state["rs"] = dpool.tile([128, 4096], FP, name="rs", tag="rs")
                    res_state["first_g"] = g
            jj = r
            for t in (0, 1):
                nc.tensor.matmul(
                    res_state["tile"][32 * jj:32 * jj + 3, t * 256:(t + 1) * 256],
                    lhsT=vc[:][:, t, n, :],
                    rhs=e[:][:, s * 512 + t * 256: s * 512 + (t + 1) * 256],
                    start=True, stop=True,
                    tile_position=(0, 32 * jj),
                )
            res_state["count"] += 1
            if res_state["count"] == 4:
                m = res_state["nflush"]
                nc.vector.tensor_copy(
                    res_state["rs"][:, m * 512:(m + 1) * 512], res_state["tile"][:]
                )
                res_state["count"] = 0
                res_state["tile"] = None
                res_state["nflush"] += 1
                if res_state["nflush"] == 8:
                    division_batch()

    def division_batch():
        b0 = res_state["first_g"] // N
        rs = res_state["rs"]
        # rows {32j+r} of rs -> DRAM scratch already in dense layout:
        # scr[4m+j, r*512 + tq] ; then scratch -> dn is a contiguous copy.
        scr = drs.tile([N, 1536], FP, name="scr", tag="scr")
        rsv = rs[:].rearrange("(j p2) (m tq) -> j p2 m tq", j=4, m=8)
        sw = scr[:].rearrange("(m j) (r tq) -> j m r tq", m=8, r=3)
        for r in (0, 1, 2):
            nc.sync.dma_start(out=sw[:, :, r, :], in_=rsv[:, r, :, :])
        # scratch -> dense [32, 1536]: partition 4m+j (= local pair n), free (r,t,q)
        dn = dpool.tile([N, 1536], FP, name="dn", tag="dn")
        nc.sync.dma_start(out=dn[:], in_=scr[:])
        dnv = dn[:].rearrange("p (r t q) -> p r t q", r=3, t=2)
        den = dpool.tile([N, BS], FP, name="den", tag="den")
        num = dpool.tile([N, BS], FP, name="num", tag="num")
        nc.vector.tensor_add(den[:], dnv[:, 0, 0, :], dnv[:, 0, 1, :])
        nc.vector.tensor_add(num[:], dnv[:, 1, 0, :], dnv[:, 1, 1, :])
        nc.vector.tensor_add(num[:], num[:], dnv[:, 2, 0, :])
        nc.vector.tensor_add(num[:], num[:], dnv[:, 2, 1, :])
        nc.vector.reciprocal_approx_fast(out=den[:], in_=den[:])
        ov = dpool.tile([N, BS], FP, name="ov", tag="ov")
        nc.vector.tensor_mul(ov[:], num[:], den[:])
        nc.vector.tensor_scalar(
            out=ov[:], in0=ov[:], scalar1=sc["wvo"], scalar2=sc["bvo"] + sc["bo"],
            op0=ALU.mult, op1=ALU.add,
        )
        nc.sync.dma_start(out=out_t[:, b0 * BS:(b0 + 1) * BS], in_=ov[:])
        res_state["nflush"] = 0
        res_state["rs"] = None

    pending = None
    cur_stage = None
    cur_rows = None
    members = []
    for g in range(PAIRS):
        b, n = divmod(g, N)
        if n == 0:
            vcur[0] = load_vcomb(b)
        if n % 16 == 0:
            cur_rows = stage_rows(b, n // 16)
        qks = cur_rows
        nn = n % 16
        s = g % GROUP
        if s == 0:
            members = []
            cur_stage = ps_stage.tile([128, GROUP * 512], FP, name="st", tag="st")
        for t in (0, 1):
            lhsT = qks[:][0:1, (2 * nn + 1) * BS + t * 128: (2 * nn + 1) * BS + (t + 1) * 128]
            rhs = qks[:][0:1, (2 * nn) * BS: (2 * nn + 1) * BS]
            nc.tensor.matmul(
                cur_stage[:, s * 512 + t * 256: s * 512 + (t + 1) * 256],
                lhsT=lhsT, rhs=rhs,
                start=True, stop=True,
                tile_position=(0, 0),
            )
        members.append((s, b, n, vcur[0]))
        if s == GROUP - 1 or g == PAIRS - 1:
            e = epool.tile([128, GROUP * 512], EDT, name="e", tag="e")
            width = len(members) * 512
            nc.scalar.activation(e[:][:, 0:width], cur_stage[:][:, 0:width], AF.Exp)
            if pending is not None:
                emit_reduces(pending)
            pending = (e, members)
    emit_reduces(pending)
    assert res_state["count"] == 0 and res_state["nflush"] == 0, (
        "pair count must be a multiple of 32 (one block per division batch)"
    )


_CACHE: dict = {}


def _get_nc(sc, reps: int = 1) -> bass.Bass:
    key = (tuple(sorted(sc.items())), reps)
    if key not in _CACHE:
        _CACHE[key] = build_kernel_module(sc, reps)
    return _CACHE[key]


def make_in_maps(query, key, value, in_proj_w, in_proj_b, out_proj_w, out_proj_b):
    q = np.ascontiguousarray(np.asarray(query, dtype=np.float32).reshape(L, N))
    k = np.ascontiguousarray(np.asarray(key, dtype=np.float32).reshape(L, N))
    vv = np.ascontiguousarray(np.asarray(value, dtype=np.float32).reshape(L, N))
    wq, wk, wv = [float(x) for x in np.asarray(in_proj_w, dtype=np.float32).reshape(3)]
    bq, bk, bv = [float(x) for x in np.asarray(in_proj_b, dtype=np.float32).reshape(3)]
    wo = float(np.asarray(out_proj_w, dtype=np.float32).reshape(1)[0])
    bo = float(np.asarray(out_proj_b, dtype=np.float32).reshape(1)[0])
    sc = {"wq": wq, "bq": bq, "wk": wk, "bk": bk,
          "wvo": float(np.float32(wo) * np.float32(wv)),
          "bvo": float(np.float32(wo) * np.float32(bv)), "bo": bo}
    in_maps = []
    for c in range(NCORES):
        sl = slice(c * LS, (c + 1) * LS)
        qr = np.ascontiguousarray(q[sl].T).reshape(N, 4, LS // 4)
        kr = np.ascontiguousarray(k[sl].T).reshape(N, 4, LS // 4)
        qkt_np = np.concatenate([qr, kr], axis=2).reshape(128, 2048)
        in_maps.append({
            "qkt": np.ascontiguousarray(qkt_np),
            "v": np.ascontiguousarray(vv[sl]),
        })
    return in_maps, sc


def run(in_maps, sc, **kwargs):
    return run_bass_kernel_spmd(_get_nc(sc), in_maps, list(range(NCORES)), **kwargs)


def assemble(results) -> np.ndarray:
    outs = [np.asarray(results[c]["out_t"], dtype=np.float32).T for c in range(NCORES)]
    return np.ascontiguousarray(np.concatenate(outs, axis=0)).reshape(L, N, 1)


def kernel(query, key, value, in_proj_w, in_proj_b, out_proj_w, out_proj_b):
    in_maps, sc = make_in_maps(
        query, key, value, in_proj_w, in_proj_b, out_proj_w, out_proj_b
    )
    res = run(in_maps, sc)
    return assemble(res.results)


# revision 7
# speedup vs baseline: 1.0112x; 1.0112x over previous
"""Trainium2 Bass kernel for nn_BlockCrossAttn (block-diagonal attention, E=H=1).

Math per (block b, batch n) pair (256-long vectors q', k', v of the block):
    q' = wq*Q + bq ; k' = wk*K + bk
    soft[q,k] = softmax_k(q'[q] * k'[k])
    out[q] = wvo * (sum_k soft[q,k] * V[k]) + (bvo + bo)
where wvo = wo*wv, bvo = wo*bv (the V/out affine folds into the epilogue
because softmax weights sum to 1).

Sharding: 128 blocks of 256 rows; 16 blocks per core across 8 cores
(fully independent, no collectives).

Per-core device pipeline (512 pairs, 171 groups of <=3):
  - Host prepacks fp16 q'/k' per pair as [2, 640] = [q|0|k0 ; 0|q|k1]:
    ONE K=2 PE matmul per pair builds S^T[k, (t,q)] = [128, 512] into a
    single PSUM bank (the K=1 outer-product form is fixed-cost bound at
    ~230ns per 256 cols; this streams 512 cols in one go).
  - ScalarE exp over [128, 1536] 3-bank PSUM spans -> E (fp16) in SBUF.
    Every 8th group instead goes to VectorE as a two-term Schraudolph exp:
    E' = bitcast_f16(i16(s*1024*log2e + B1)) + sqrt2 *
         bitcast_f16(i16(s*1024*log2e + B2))  (~0.8% rel err; the common
    scale cancels in the softmax ratio).
  - PE reduction matmuls: lhsT = [ones, v_hi, v_lo] fp16 (host-packed),
    rhs = E; the two k-half matmuls accumulate into one PSUM [3, 256]
    region (start/stop), 8 pairs per result bank.
  - VectorE flushes banks to SBUF; a DRAM scratch bounce re-lays 32 pairs
    into a dense [32, 768] tile; num = hi+lo, reciprocal_approx_fast +
    multiply + affine epilogue; one contiguous DMA per block to the
    n-major output.

Weight scalars are baked into the module as immediates (compiled per
weight set, cached).
"""

from contextlib import ExitStack

import numpy as np

import concourse.bacc as bacc
import concourse.bass as bass
import concourse.tile as tile
from concourse import mybir
from concourse.bass_utils import run_bass_kernel_spmd

FP = mybir.dt.float32
F16 = mybir.dt.float16
I16 = mybir.dt.int16
AF = mybir.ActivationFunctionType
ALU = mybir.AluOpType

L = 32768          # sequence length
N = 32             # batch
BS = 256           # block size
NB = L // BS       # 128 blocks
NCORES = 8
BPC = NB // NCORES  # 16 blocks per core
LS = BPC * BS       # 4096 rows per core shard

GROUP = 3           # pairs per PSUM stage group (3 banks)
PAIRS = BPC * N     # 512 pairs per core
PW = 640            # qz cols per pair: rhs 512 | lhsT 128
SPD = 16            # pairs per staging DMA

# Schraudolph constants (fp16-domain, int16-bitcast), calibrated for
# round-to-nearest fp32->int16 conversion; E' = a + sqrt2*b.
SCHR_SCALE = float(1024.0 * np.log2(np.e))
SCHR_B1 = 15360.0 - 55.0
SCHR_B2 = 15360.0 - 512.0 - 56.0
SQRT2 = float(np.sqrt(2.0))
SCHR_EVERY = 10 ** 9  # every k-th group exps on DVE instead of ACT (disabled)


def build_kernel_module(sc, reps: int = 1) -> bass.Bass:
    nc = bacc.Bacc("TRN2", target_bir_lowering=False, debug=False, num_devices=NCORES)
    qz = nc.declare_dram_parameter("qz", [PAIRS, 2, PW], F16, isOutput=False)
    vz = nc.declare_dram_parameter("vz", [BPC, 128, 2 * N * 3], F16, isOutput=False)
    out_t = nc.declare_dram_parameter("out_t", [N, LS], FP, isOutput=True)

    with tile.TileContext(nc) as tc:
        with ExitStack() as ctx:
            if reps == 1:
                _emit(ctx, tc, qz, vz, out_t, sc)
            else:
                with tc.For_i(0, reps, 1):
                    _emit(ctx, tc, qz, vz, out_t, sc)
    nc.compile()
    return nc


def _emit(ctx, tc, qz, vz, out_t, sc):
    nc = tc.nc

    stage = ctx.enter_context(tc.tile_pool(name="stage", bufs=2))
    vpool = ctx.enter_context(tc.tile_pool(name="vpool", bufs=2))
    epool = ctx.enter_context(tc.tile_pool(name="epool", bufs=3))
    spool = ctx.enter_context(tc.tile_pool(name="spool", bufs=2))
    dpool = ctx.enter_context(tc.tile_pool(name="dpool", bufs=2))
    ps_stage = ctx.enter_context(tc.tile_pool(name="ps_stage", bufs=2, space="PSUM"))
    ps_res = ctx.enter_context(tc.tile_pool(name="ps_res", bufs=2, space="PSUM"))
    drs = ctx.enter_context(tc.tile_pool(name="drs", bufs=2, space="DRAM"))

    def load_qz(c):
        qk = stage.tile([2, SPD * PW], F16, name="qk", tag="qk")
        nc.sync.dma_start(
            out=qk[:].rearrange("p (s w) -> p s w", s=SPD),
            in_=qz[c * SPD:(c + 1) * SPD].rearrange("s p w -> p s w"),
        )
        return qk

    def load_vz(b):
        vc = vpool.tile([128, 2, N, 3], F16, name="vc", tag="vc")
        nc.sync.dma_start(out=vc[:].rearrange("p t n c -> p (t n c)"), in_=vz[b])
        return vc

    # --- reduction / division ---------------------------------------------------
    res_state = {"tile": None, "count": 0, "nflush": 0, "rs": None, "first_g": 0}

    def emit_reduces(pend):
        e, members = pend
        for (s, b, n, vc) in members:
            g = b * N + n
            p8 = res_state["count"]
            if p8 == 0:
                res_state["tile"] = ps_res.tile([128, 512], FP, name="res", tag="res")
                if res_state["nflush"] == 0:
                    res_state["rs"] = dpool.tile([128, 2048], FP, name="rs", tag="rs")
                    res_state["first_g"] = g
            j, h = p8 % 4, p8 // 4
            for t in (0, 1):
                nc.tensor.matmul(
                    res_state["tile"][32 * j:32 * j + 3, h * 256:(h + 1) * 256],
                    lhsT=vc[:][:, t, n, :],
                    rhs=e[:][:, s * 512 + t * 256: s * 512 + (t + 1) * 256],
                    start=(t == 0), stop=(t == 1),
                    tile_position=(0, 32 * j),
                )
            res_state["count"] += 1
            if res_state["count"] == 8:
                m = res_state["nflush"]
                nc.vector.tensor_copy(
                    res_state["rs"][:, m * 512:(m + 1) * 512], res_state["tile"][:]
                )
                res_state["count"] = 0
                res_state["tile"] = None
                res_state["nflush"] += 1
                if res_state["nflush"] == 4:
                    division_batch()

    def division_batch():
        b0 = res_state["first_g"] // N
        rs = res_state["rs"]
        # rows {32j+r} of rs (pair n = m*8 + h*4 + j at cols m*512+h*256)
        # -> DRAM scratch in dense layout scr[n, r*256+q]
        scr = drs.tile([N, 768], FP, name="scr", tag="scr")
        rsv = rs[:].rearrange("(j p2) (m h q) -> j p2 m h q", j=4, m=4, h=2)
        sw = scr[:].rearrange("(m h j) (r q) -> j m h r q", m=4, h=2, r=3)
        for r in (0, 1, 2):
            nc.sync.dma_start(out=sw[:, :, :, r, :], in_=rsv[:, r, :, :, :])
        dn = dpool.tile([N, 768], FP, name="dn", tag="dn")
        nc.sync.dma_start(out=dn[:], in_=scr[:])
        dnv = dn[:].rearrange("p (r q) -> p r q", r=3)
        num = dpool.tile([N, BS], FP, name="num", tag="num")
        den = dpool.tile([N, BS], FP, name="den", tag="den")
        nc.vector.tensor_add(num[:], dnv[:, 1, :], dnv[:, 2, :])
        nc.vector.reciprocal_approx_fast(out=den[:], in_=dnv[:, 0, :])
        ov = dpool.tile([N, BS], FP, name="ov", tag="ov")
        nc.vector.tensor_mul(ov[:], num[:], den[:])
        nc.vector.tensor_scalar(
            out=ov[:], in0=ov[:], scalar1=sc["wvo"], scalar2=sc["bvo"] + sc["bo"],
            op0=ALU.mult, op1=ALU.add,
        )
        nc.sync.dma_start(out=out_t[:, b0 * BS:(b0 + 1) * BS], in_=ov[:])
        res_state["nflush"] = 0
        res_state["rs"] = None

    # --- main loop --------------------------------------------------------------
    pending = None
    cur_qk = None
    vcur = [None]
    members = []
    st = None
    for g in range(PAIRS):
        b, n = divmod(g, N)
        if n == 0:
            vcur[0] = load_vz(b)
        if g % SPD == 0:
            cur_qk = load_qz(g // SPD)
        s = g % GROUP
        if s == 0:
            members = []
            st = ps_stage.tile([128, GROUP * 512], FP, name="st", tag="st")
        sl = (g % SPD) * PW
        nc.tensor.matmul(
            st[:, s * 512:(s + 1) * 512],
            lhsT=cur_qk[:][:, sl + 512: sl + PW],
            rhs=cur_qk[:][:, sl: sl + 512],
            start=True, stop=True,
            tile_position=(0, 0),
        )
        members.append((s, b, n, vcur[0]))
        if s == GROUP - 1 or g == PAIRS - 1:
            width = len(members) * 512
            e = epool.tile([128, GROUP * 512], F16, name="e", tag="e")
            if (g // GROUP) % SCHR_EVERY == SCHR_EVERY // 2:
                a16 = spool.tile([128, GROUP * 512], I16, name="a16", tag="a16")
                b16 = spool.tile([128, GROUP * 512], I16, name="b16", tag="b16")
                nc.vector.tensor_scalar(
                    out=a16[:, 0:width], in0=st[:][:, 0:width],
                    scalar1=SCHR_SCALE, scalar2=SCHR_B1, op0=ALU.mult, op1=ALU.add,
                )
                nc.vector.tensor_scalar(
                    out=b16[:, 0:width], in0=st[:][:, 0:width],
                    scalar1=SCHR_SCALE, scalar2=SCHR_B2, op0=ALU.mult, op1=ALU.add,
                )
                nc.vector.affine_then_add(
                    out=e[:][:, 0:width],
                    in0=b16[:].bitcast(F16)[:, 0:width],
                    in1=a16[:].bitcast(F16)[:, 0:width],
                    scale=SQRT2, bias=0.0,
                )
            else:
                nc.scalar.activation(e[:][:, 0:width], st[:][:, 0:width], AF.Exp)
            if pending is not None:
                emit_reduces(pending)
            pending = (e, members)
    emit_reduces(pending)
    assert res_state["count"] == 0 and res_state["nflush"] == 0, (
        "pair count must be a multiple of 32 (one block per division batch)"
    )


_CACHE: dict = {}


def _get_nc(sc, reps: int = 1) -> bass.Bass:
    key = (tuple(sorted(sc.items())), reps)
    if key not in _CACHE:
        _CACHE[key] = build_kernel_module(sc, reps)
    return _CACHE[key]


def make_in_maps(query, key, value, in_proj_w, in_proj_b, out_proj_w, out_proj_b):
    q = np.ascontiguousarray(np.asarray(query, dtype=np.float32).reshape(L, N))
    k = np.ascontiguousarray(np.asarray(key, dtype=np.float32).reshape(L, N))
    vv = np.ascontiguousarray(np.asarray(value, dtype=np.float32).reshape(L, N))
    wq, wk, wv = [float(x) for x in np.asarray(in_proj_w, dtype=np.float32).reshape(3)]
    bq, bk, bv = [float(x) for x in np.asarray(in_proj_b, dtype=np.float32).reshape(3)]
    wo = float(np.asarray(out_proj_w, dtype=np.float32).reshape(1)[0])
    bo = float(np.asarray(out_proj_b, dtype=np.float32).reshape(1)[0])
    sc = {"wvo": float(np.float32(wo) * np.float32(wv)),
          "bvo": float(np.float32(wo) * np.float32(bv)), "bo": bo}

    # host-side fp16 projections
    q16 = (q * np.float32(wq) + np.float32(bq)).astype(np.float16)
    k16 = (k * np.float32(wk) + np.float32(bk)).astype(np.float16)
    vhi = vv.astype(np.float16)
    vlo = (vv - vhi.astype(np.float32)).astype(np.float16)

    p = np.arange(PAIRS)
    b, n = p // N, p % N
    ar = np.arange(256)
    ar128 = np.arange(128)

    in_maps = []
    for c in range(NCORES):
        sl = slice(c * LS, (c + 1) * LS)
        qc = np.ascontiguousarray(q16[sl].T)   # [N, LS]
        kc = np.ascontiguousarray(k16[sl].T)
        qrows = qc[n[:, None], (b * BS)[:, None] + ar]            # [512, 256]
        qzc = np.zeros((PAIRS, 2, PW), np.float16)
        qzc[:, 0, 0:256] = qrows
        qzc[:, 1, 256:512] = qrows
        for t in (0, 1):
            qzc[:, t, 512:640] = kc[n[:, None], (b * BS + t * 128)[:, None] + ar128]
        # vz[b, p, (t, n, c)]
        vzc = np.empty((BPC, 128, 2, N, 3), np.float16)
        vzc[:, :, :, :, 0] = 1.0
        vzc[:, :, :, :, 1] = vhi[sl].reshape(BPC, 2, 128, N).transpose(0, 2, 1, 3)
        vzc[:, :, :, :, 2] = vlo[sl].reshape(BPC, 2, 128, N).transpose(0, 2, 1, 3)
        in_maps.append({
            "qz": np.ascontiguousarray(qzc),
            "vz": np.ascontiguousarray(vzc.reshape(BPC, 128, 2 * N * 3)),
        })
    return in_maps, sc


def run(in_maps, sc, **kwargs):
    return run_bass_kernel_spmd(_get_nc(sc), in_maps, list(range(NCORES)), **kwargs)


def assemble(results) -> np.ndarray:
    outs = [np.asarray(results[c]["out_t"], dtype=np.float32).T for c in range(NCORES)]
    return np.ascontiguousarray(np.concatenate(outs, axis=0)).reshape(L, N, 1)


def kernel(query, key, value, in_proj_w, in_proj_b, out_proj_w, out_proj_b):
    in_maps, sc = make_in_maps(
        query, key, value, in_proj_w, in_proj_b, out_proj_w, out_proj_b
    )
    res = run(in_maps, sc)
    return assemble(res.results)


# revision 10
# speedup vs baseline: 1.0609x; 1.0492x over previous
"""Trainium2 Bass kernel for nn_BlockCrossAttn (block-diagonal attention, E=H=1).

Math per (block b, batch n) pair (256-long vectors q', k', v of the block):
    q' = wq*Q + bq ; k' = wk*K + bk
    soft[q,k] = softmax_k(q'[q] * k'[k])
    out[q] = wvo * (sum_k soft[q,k] * V[k]) + (bvo + bo)

Sharding: 128 blocks of 256 rows; 16 blocks per core across 8 cores.

Per-core pipeline (512 pairs, 171 groups of <=3).  Scores S^T[k, (t,q)] are
PSUM-drain-bound on the PE (~0.9ns/col regardless of matmul shape), and exp
is ScalarE-bound (~0.96ns/col), so groups are split across three classes to
balance all four engines:

  - class A (default): one K=2 PE matmul per pair (host-packed zero-padded
    [2, 640] = [q|0|k0 ; 0|q|k1] fp16 operands) -> PSUM bank; ScalarE exp
    [128, 1536] -> E fp16 in SBUF.
  - class C (PE relief, g%8 in {1,3,5}): GpSimd partition-broadcasts the q
    row; VectorE tensor_scalar (per-partition k scalar AP) builds the score
    span in SBUF at 2-byte rate; ScalarE exp from SBUF.
  - class B (ScalarE relief, g%8==7): PE scores as in A, then VectorE
    computes a two-term Schraudolph exp E = f16^(i16(s*1024*log2e + B1)) +
    sqrt2 * f16^(i16(s*1024*log2e + B2)) (~0.8% rel err, common scale
    cancels in softmax).  The sqrt2 combine is folded into the reduction:
    the second term uses a sqrt2-scaled [ones,v_hi,v_lo] triplet and
    accumulates into the same PSUM region (4 matmuls instead of 2).

  - Reduction: lhsT = [ones, v_hi, v_lo] fp16 (host-packed), rhs = E; the
    k-half matmuls accumulate into one PSUM [3, 256] region, 8 pairs per
    result bank.  VectorE flushes banks; a DRAM scratch bounce re-lays 32
    pairs into [32, 768]; num = hi+lo, reciprocal_approx_fast, multiply,
    affine epilogue; one contiguous DMA per block to the n-major output.
"""

from contextlib import ExitStack

import numpy as np

import concourse.bacc as bacc
import concourse.bass as bass
import concourse.tile as tile
from concourse import mybir
from concourse.bass_utils import run_bass_kernel_spmd

FP = mybir.dt.float32
F16 = mybir.dt.float16
I16 = mybir.dt.int16
AF = mybir.ActivationFunctionType
ALU = mybir.AluOpType

L = 32768          # sequence length
N = 32             # batch
BS = 256           # block size
NB = L // BS       # 128 blocks
NCORES = 8
BPC = NB // NCORES  # 16 blocks per core
LS = BPC * BS       # 4096 rows per core shard

GROUP = 3           # pairs per PSUM stage group (3 banks)
PAIRS = BPC * N     # 512 pairs per core
PW = 640            # qz cols per pair: rhs 512 | lhsT 128
SPD = 16            # pairs per staging DMA

# Schraudolph constants (fp16-domain, int16-bitcast, +0.25 hedges the
# rint-vs-trunc convert ambiguity); E = a + sqrt2*b via the dual triplet.
SCHR_SCALE = float(1024.0 * np.log2(np.e))
SCHR_B1 = 15305.25
SCHR_B2 = 14792.25
SQRT2 = float(np.sqrt(2.0))


def group_class(g):
    if g % 8 in (1, 3, 5):
        return "C"
    if g % 8 == 7:
        return "B"
    return "A"


def build_kernel_module(sc, reps: int = 1) -> bass.Bass:
    nc = bacc.Bacc("TRN2", target_bir_lowering=False, debug=False, num_devices=NCORES)
    qz = nc.declare_dram_parameter("qz", [PAIRS, 2, PW], F16, isOutput=False)
    vz = nc.declare_dram_parameter("vz", [BPC, 128, 2 * N * 6], F16, isOutput=False)
    kz = nc.declare_dram_parameter("kz", [BPC, 128, 2 * N], FP, isOutput=False)
    out_t = nc.declare_dram_parameter("out_t", [N, LS], FP, isOutput=True)

    with tile.TileContext(nc) as tc:
        with ExitStack() as ctx:
            if reps == 1:
                _emit(ctx, tc, qz, vz, kz, out_t, sc)
            else:
                with tc.For_i(0, reps, 1):
                    _emit(ctx, tc, qz, vz, kz, out_t, sc)
    nc.compile()
    return nc


def _emit(ctx, tc, qz, vz, kz, out_t, sc):
    nc = tc.nc

    stage = ctx.enter_context(tc.tile_pool(name="stage", bufs=2))
    vpool = ctx.enter_context(tc.tile_pool(name="vpool", bufs=2))
    kpool = ctx.enter_context(tc.tile_pool(name="kpool", bufs=2))
    qpool = ctx.enter_context(tc.tile_pool(name="qpool", bufs=4))
    epool = ctx.enter_context(tc.tile_pool(name="epool", bufs=3))
    spool = ctx.enter_context(tc.tile_pool(name="spool", bufs=4))
    dpool = ctx.enter_context(tc.tile_pool(name="dpool", bufs=2))
    ps_stage = ctx.enter_context(tc.tile_pool(name="ps_stage", bufs=2, space="PSUM"))
    ps_res = ctx.enter_context(tc.tile_pool(name="ps_res", bufs=2, space="PSUM"))
    drs = ctx.enter_context(tc.tile_pool(name="drs", bufs=2, space="DRAM"))

    def load_qz(c):
        qk = stage.tile([2, SPD * PW], F16, name="qk", tag="qk")
        nc.sync.dma_start(
            out=qk[:].rearrange("p (s w) -> p s w", s=SPD),
            in_=qz[c * SPD:(c + 1) * SPD].rearrange("s p w -> p s w"),
        )
        return qk

    def load_vz(b):
        vc = vpool.tile([128, 2, N, 6], F16, name="vc", tag="vc")
        nc.sync.dma_start(out=vc[:].rearrange("p t n c -> p (t n c)"), in_=vz[b])
        kc = kpool.tile([128, 2, N], FP, name="kc", tag="kc")
        nc.sync.dma_start(out=kc[:].rearrange("p t n -> p (t n)"), in_=kz[b])
        return vc, kc

    # --- reduction / division ---------------------------------------------------
    res_state = {"tile": None, "count": 0, "nflush": 0, "rs": None, "first_g": 0}

    def emit_reduces(pend):
        rhs_tiles, members = pend
        for (s, b, n, vc) in members:
            g = b * N + n
            p8 = res_state["count"]
            if p8 == 0:
                res_state["tile"] = ps_res.tile([128, 512], FP, name="res", tag="res")
                if res_state["nflush"] == 0:
                    res_state["rs"] = dpool.tile([128, 2048], FP, name="rs", tag="rs")
                    res_state["first_g"] = g
            j, h = p8 % 4, p8 // 4
            nmm = len(rhs_tiles) * 2
            i = 0
            for (e, c0) in rhs_tiles:
                for t in (0, 1):
                    nc.tensor.matmul(
                        res_state["tile"][32 * j:32 * j + 3, h * 256:(h + 1) * 256],
                        lhsT=vc[:][:, t, n, c0:c0 + 3],
                        rhs=e[:][:, s * 512 + t * 256: s * 512 + (t + 1) * 256],
                        start=(i == 0), stop=(i == nmm - 1),
                        tile_position=(0, 32 * j),
                    )
                    i += 1
            res_state["count"] += 1
            if res_state["count"] == 8:
                m = res_state["nflush"]
                nc.vector.tensor_copy(
                    res_state["rs"][:, m * 512:(m + 1) * 512], res_state["tile"][:]
                )
                res_state["count"] = 0
                res_state["tile"] = None
                res_state["nflush"] += 1
                if res_state["nflush"] == 4:
                    division_batch()

    def division_batch():
        b0 = res_state["first_g"] // N
        rs = res_state["rs"]
        scr = drs.tile([N, 768], FP, name="scr", tag="scr")
        rsv = rs[:].rearrange("(j p2) (m h q) -> j p2 m h q", j=4, m=4, h=2)
        sw = scr[:].rearrange("(m h j) (r q) -> j m h r q", m=4, h=2, r=3)
        for r in (0, 1, 2):
            nc.sync.dma_start(out=sw[:, :, :, r, :], in_=rsv[:, r, :, :, :])
        dn = dpool.tile([N, 768], FP, name="dn", tag="dn")
        nc.sync.dma_start(out=dn[:], in_=scr[:])
        dnv = dn[:].rearrange("p (r q) -> p r q", r=3)
        num = dpool.tile([N, BS], FP, name="num", tag="num")
        den = dpool.tile([N, BS], FP, name="den", tag="den")
        nc.vector.tensor_add(num[:], dnv[:, 1, :], dnv[:, 2, :])
        nc.vector.reciprocal_approx_fast(out=den[:], in_=dnv[:, 0, :])
        ov = dpool.tile([N, BS], FP, name="ov", tag="ov")
        nc.vector.tensor_mul(ov[:], num[:], den[:])
        nc.vector.tensor_scalar(
            out=ov[:], in0=ov[:], scalar1=sc["wvo"], scalar2=sc["bvo"] + sc["bo"],
            op0=ALU.mult, op1=ALU.add,
        )
        nc.sync.dma_start(out=out_t[:, b0 * BS:(b0 + 1) * BS], in_=ov[:])
        res_state["nflush"] = 0
        res_state["rs"] = None

    # --- main loop --------------------------------------------------------------
    pending = None
    cur_qk = None
    vcur = [None]
    kcur = [None]
    NGRP = (PAIRS + GROUP - 1) // GROUP
    for g in range(NGRP):
        cls = group_class(g)
        p0 = g * GROUP
        npair = min(GROUP, PAIRS - p0)
        width = npair * 512
        members = []
        st = None
        sc16 = None
        for s in range(npair):
            p = p0 + s
            b, n = divmod(p, N)
            if n == 0:
                vcur[0], kcur[0] = load_vz(b)
            if p % SPD == 0:
                cur_qk = load_qz(p // SPD)
            sl = (p % SPD) * PW
            if cls == "C":
                if s == 0:
                    sc16 = spool.tile([128, GROUP * 512], F16, name="sc16", tag="sc16")
                qb = qpool.tile([128, 256], F16, name="qb", tag="qb")
                nc.gpsimd.partition_broadcast(qb[:], cur_qk[:][0:1, sl:sl + 256])
                for t in (0, 1):
                    nc.vector.tensor_scalar(
                        out=sc16[:, s * 512 + t * 256: s * 512 + (t + 1) * 256],
                        in0=qb[:], scalar1=kcur[0][:][:, t, n:n + 1],
                        scalar2=None, op0=ALU.mult, op1=ALU.bypass,
                    )
            else:
                if s == 0:
                    st = ps_stage.tile([128, GROUP * 512], FP, name="st", tag="st")
                nc.tensor.matmul(
                    st[:, s * 512:(s + 1) * 512],
                    lhsT=cur_qk[:][:, sl + 512: sl + PW],
                    rhs=cur_qk[:][:, sl: sl + 512],
                    start=True, stop=True,
                    tile_position=(0, 0),
                )
            members.append((s, b, n, vcur[0]))

        if cls == "B":
            a16 = spool.tile([128, GROUP * 512], I16, name="a16", tag="a16")
            b16 = spool.tile([128, GROUP * 512], I16, name="b16", tag="b16")
            nc.vector.tensor_scalar(
                out=a16[:, 0:width], in0=st[:][:, 0:width],
                scalar1=SCHR_SCALE, scalar2=SCHR_B1, op0=ALU.mult, op1=ALU.add,
            )
            nc.vector.tensor_scalar(
                out=b16[:, 0:width], in0=st[:][:, 0:width],
                scalar1=SCHR_SCALE, scalar2=SCHR_B2, op0=ALU.mult, op1=ALU.add,
            )
            rhs_tiles = [(a16[:].bitcast(F16), 0), (b16[:].bitcast(F16), 3)]
        else:
            e = epool.tile([128, GROUP * 512], F16, name="e", tag="e")
            src = sc16[:] if cls == "C" else st[:]
            nc.scalar.activation(e[:][:, 0:width], src[:, 0:width], AF.Exp)
            rhs_tiles = [(e[:], 0)]
        if pending is not None:
            emit_reduces(pending)
        pending = (rhs_tiles, members)
    emit_reduces(pending)
    assert res_state["count"] == 0 and res_state["nflush"] == 0, (
        "pair count must be a multiple of 32 (one block per division batch)"
    )


_CACHE: dict = {}


def _get_nc(sc, reps: int = 1) -> bass.Bass:
    key = (tuple(sorted(sc.items())), reps)
    if key not in _CACHE:
        _CACHE[key] = build_kernel_module(sc, reps)
    return _CACHE[key]


def make_in_maps(query, key, value, in_proj_w, in_proj_b, out_proj_w, out_proj_b):
    q = np.ascontiguousarray(np.asarray(query, dtype=np.float32).reshape(L, N))
    k = np.ascontiguousarray(np.asarray(key, dtype=np.float32).reshape(L, N))
    vv = np.ascontiguousarray(np.asarray(value, dtype=np.float32).reshape(L, N))
    wq, wk, wv = [float(x) for x in np.asarray(in_proj_w, dtype=np.float32).reshape(3)]
    bq, bk, bv = [float(x) for x in np.asarray(in_proj_b, dtype=np.float32).reshape(3)]
    wo = float(np.asarray(out_proj_w, dtype=np.float32).reshape(1)[0])
    bo = float(np.asarray(out_proj_b, dtype=np.float32).reshape(1)[0])
    sc = {"wvo": float(np.float32(wo) * np.float32(wv)),
          "bvo": float(np.float32(wo) * np.float32(bv)), "bo": bo}

    q16 = (q * np.float32(wq) + np.float32(bq)).astype(np.float16)
    k16 = (k * np.float32(wk) + np.float32(bk)).astype(np.float16)
    vhi = vv.astype(np.float16)
    vlo = (vv - vhi.astype(np.float32)).astype(np.float16)

    p = np.arange(PAIRS)
    b, n = p // N, p % N
    ar = np.arange(256)
    ar128 = np.arange(128)

    in_maps = []
    for c in range(NCORES):
        sl = slice(c * LS, (c + 1) * LS)
        qc = np.ascontiguousarray(q16[sl].T)   # [N, LS]
        kc = np.ascontiguousarray(k16[sl].T)
        qrows = qc[n[:, None], (b * BS)[:, None] + ar]            # [512, 256]
        qzc = np.zeros((PAIRS, 2, PW), np.float16)
        qzc[:, 0, 0:256] = qrows
        qzc[:, 1, 256:512] = qrows
        for t in (0, 1):
            qzc[:, t, 512:640] = kc[n[:, None], (b * BS + t * 128)[:, None] + ar128]
        # vz[b, p, (t, n, c6)]: cols 0:3 = (1, vhi, vlo), 3:6 = sqrt2 * same
        vzc = np.empty((BPC, 128, 2, N, 6), np.float16)
        vzc[:, :, :, :, 0] = 1.0
        vzc[:, :, :, :, 1] = vhi[sl].reshape(BPC, 2, 128, N).transpose(0, 2, 1, 3)
        vzc[:, :, :, :, 2] = vlo[sl].reshape(BPC, 2, 128, N).transpose(0, 2, 1, 3)
        vzc[:, :, :, :, 3:6] = (
            vzc[:, :, :, :, 0:3].astype(np.float32) * np.float32(SQRT2)
        ).astype(np.float16)
        # kz[b, p, (t, n)] = k'[b*256 + t*128 + p, n]
        kzc = np.ascontiguousarray(
            k16[sl].astype(np.float32).reshape(BPC, 2, 128, N).transpose(0, 2, 1, 3)
        )
        in_maps.append({
            "qz": np.ascontiguousarray(qzc),
            "vz": np.ascontiguousarray(vzc.reshape(BPC, 128, 2 * N * 6)),
            "kz": np.ascontiguousarray(kzc.reshape(BPC, 128, 2 * N)),
        })
    return in_maps, sc


def run(in_maps, sc, **kwargs):
    return run_bass_kernel_spmd(_get_nc(sc), in_maps, list(range(NCORES)), **kwargs)


def assemble(results) -> np.ndarray:
    outs = [np.asarray(results[c]["out_t"], dtype=np.float32).T for c in range(NCORES)]
    return np.ascontiguousarray(np.concatenate(outs, axis=0)).reshape(L, N, 1)


def kernel(query, key, value, in_proj_w, in_proj_b, out_proj_w, out_proj_b):
    in_maps, sc = make_in_maps(
        query, key, value, in_proj_w, in_proj_b, out_proj_w, out_proj_b
    )
    res = run(in_maps, sc)
    return assemble(res.results)


# revision 11
# speedup vs baseline: 1.1019x; 1.0386x over previous
"""Trainium2 Bass kernel for nn_BlockCrossAttn (block-diagonal attention, E=H=1).

Math per (block b, batch n) pair (256-long vectors q', k', v of the block):
    q' = wq*Q + bq ; k' = wk*K + bk
    soft[q,k] = softmax_k(q'[q] * k'[k])
    out[q] = wvo * (sum_k soft[q,k] * V[k]) + (bvo + bo)

Sharding: 128 blocks of 256 rows; 16 blocks per core across 8 cores.

Per-core pipeline (512 pairs, 171 groups of <=3).  Scores S^T[k, (t,q)] are
PSUM-drain-bound on the PE (~0.9ns/col regardless of matmul shape), and exp
is ScalarE-bound (~0.96ns/col), so groups are split across three classes to
balance all four engines:

  - class A (default): one K=2 PE matmul per pair (host-packed zero-padded
    [2, 640] = [q|0|k0 ; 0|q|k1] fp16 operands) -> PSUM bank; ScalarE exp
    [128, 1536] -> E fp16 in SBUF.
  - class C (PE relief, g%8 in {1,3,5}): GpSimd partition-broadcasts the q
    row; VectorE tensor_scalar (per-partition k scalar AP) builds the score
    span in SBUF at 2-byte rate; ScalarE exp from SBUF.
  - class B (ScalarE relief, g%8==7): PE scores as in A, then VectorE
    computes a two-term Schraudolph exp E = f16^(i16(s*1024*log2e + B1)) +
    sqrt2 * f16^(i16(s*1024*log2e + B2)) (~0.8% rel err, common scale
    cancels in softmax).  The sqrt2 combine is folded into the reduction:
    the second term uses a sqrt2-scaled [ones,v_hi,v_lo] triplet and
    accumulates into the same PSUM region (4 matmuls instead of 2).

  - Reduction: lhsT = [ones, v_hi, v_lo] fp16 (host-packed), rhs = E; the
    k-half matmuls accumulate into one PSUM [3, 256] region, 8 pairs per
    result bank.  VectorE flushes banks; a DRAM scratch bounce re-lays 32
    pairs into [32, 768]; num = hi+lo, reciprocal_approx_fast, multiply,
    affine epilogue; one contiguous DMA per block to the n-major output.
"""

from contextlib import ExitStack

import numpy as np

import concourse.bacc as bacc
import concourse.bass as bass
import concourse.tile as tile
from concourse import mybir
from concourse.bass_utils import run_bass_kernel_spmd

FP = mybir.dt.float32
F16 = mybir.dt.float16
I16 = mybir.dt.int16
AF = mybir.ActivationFunctionType
ALU = mybir.AluOpType

L = 32768          # sequence length
N = 32             # batch
BS = 256           # block size
NB = L // BS       # 128 blocks
NCORES = 8
BPC = NB // NCORES  # 16 blocks per core
LS = BPC * BS       # 4096 rows per core shard

GROUP = 3           # pairs per PSUM stage group (3 banks)
PAIRS = BPC * N     # 512 pairs per core
PW = 640            # qz cols per pair: rhs 512 | lhsT 128
SPD = 16            # pairs per staging DMA

# Schraudolph constants (fp16-domain, int16-bitcast, +0.25 hedges the
# rint-vs-trunc convert ambiguity); E = a + sqrt2*b via the dual triplet.
SCHR_SCALE = float(1024.0 * np.log2(np.e))
SCHR_B1 = 15305.25
SCHR_B2 = 14792.25
SQRT2 = float(np.sqrt(2.0))


def group_class(g):
    if g % 8 in (1, 3, 5):
        return "C"
    if g % 8 == 7:
        return "B"
    return "A"


def build_kernel_module(sc, reps: int = 1) -> bass.Bass:
    nc = bacc.Bacc("TRN2", target_bir_lowering=False, debug=False, num_devices=NCORES)
    qz = nc.declare_dram_parameter("qz", [PAIRS, 2, PW], F16, isOutput=False)
    vz = nc.declare_dram_parameter("vz", [BPC, 128, 2 * N * 6], F16, isOutput=False)
    kz = nc.declare_dram_parameter("kz", [BPC, 128, 2 * N], FP, isOutput=False)
    out_t = nc.declare_dram_parameter("out_t", [N, LS], FP, isOutput=True)

    with tile.TileContext(nc) as tc:
        with ExitStack() as ctx:
            if reps == 1:
                _emit(ctx, tc, qz, vz, kz, out_t, sc)
            else:
                with tc.For_i(0, reps, 1):
                    _emit(ctx, tc, qz, vz, kz, out_t, sc)
    nc.compile()
    return nc


def _emit(ctx, tc, qz, vz, kz, out_t, sc):
    nc = tc.nc

    stage = ctx.enter_context(tc.tile_pool(name="stage", bufs=2))
    vpool = ctx.enter_context(tc.tile_pool(name="vpool", bufs=2))
    kpool = ctx.enter_context(tc.tile_pool(name="kpool", bufs=2))
    qpool = ctx.enter_context(tc.tile_pool(name="qpool", bufs=8))
    epool = ctx.enter_context(tc.tile_pool(name="epool", bufs=4))
    spool = ctx.enter_context(tc.tile_pool(name="spool", bufs=6))
    dpool = ctx.enter_context(tc.tile_pool(name="dpool", bufs=2))
    ps_stage = ctx.enter_context(tc.tile_pool(name="ps_stage", bufs=2, space="PSUM"))
    ps_res = ctx.enter_context(tc.tile_pool(name="ps_res", bufs=2, space="PSUM"))
    drs = ctx.enter_context(tc.tile_pool(name="drs", bufs=2, space="DRAM"))

    def load_qz(c):
        qk = stage.tile([2, SPD * PW], F16, name="qk", tag="qk")
        nc.sync.dma_start(
            out=qk[:].rearrange("p (s w) -> p s w", s=SPD),
            in_=qz[c * SPD:(c + 1) * SPD].rearrange("s p w -> p s w"),
        )
        return qk

    def load_vz(b):
        vc = vpool.tile([128, 2, N, 6], F16, name="vc", tag="vc")
        nc.sync.dma_start(out=vc[:].rearrange("p t n c -> p (t n c)"), in_=vz[b])
        kc = kpool.tile([128, 2, N], FP, name="kc", tag="kc")
        nc.sync.dma_start(out=kc[:].rearrange("p t n -> p (t n)"), in_=kz[b])
        return vc, kc

    # --- reduction / division ---------------------------------------------------
    res_state = {"tile": None, "count": 0, "nflush": 0, "rs": None, "first_g": 0}

    def emit_reduces(pend):
        rhs_tiles, members = pend
        for (s, b, n, vc) in members:
            g = b * N + n
            p8 = res_state["count"]
            if p8 == 0:
                res_state["tile"] = ps_res.tile([128, 512], FP, name="res", tag="res")
                if res_state["nflush"] == 0:
                    res_state["rs"] = dpool.tile([128, 2048], FP, name="rs", tag="rs")
                    res_state["first_g"] = g
            j, h = p8 % 4, p8 // 4
            nmm = len(rhs_tiles) * 2
            i = 0
            for (e, c0) in rhs_tiles:
                for t in (0, 1):
                    nc.tensor.matmul(
                        res_state["tile"][32 * j:32 * j + 3, h * 256:(h + 1) * 256],
                        lhsT=vc[:][:, t, n, c0:c0 + 3],
                        rhs=e[:][:, s * 512 + t * 256: s * 512 + (t + 1) * 256],
                        start=(i == 0), stop=(i == nmm - 1),
                        tile_position=(0, 32 * j),
                    )
                    i += 1
            res_state["count"] += 1
            if res_state["count"] == 8:
                m = res_state["nflush"]
                nc.vector.tensor_copy(
                    res_state["rs"][:, m * 512:(m + 1) * 512], res_state["tile"][:]
                )
                res_state["count"] = 0
                res_state["tile"] = None
                res_state["nflush"] += 1
                if res_state["nflush"] == 4:
                    division_batch()

    def division_batch():
        b0 = res_state["first_g"] // N
        rs = res_state["rs"]
        scr = drs.tile([N, 768], FP, name="scr", tag="scr")
        rsv = rs[:].rearrange("(j p2) (m h q) -> j p2 m h q", j=4, m=4, h=2)
        sw = scr[:].rearrange("(m h j) (r q) -> j m h r q", m=4, h=2, r=3)
        for r in (0, 1, 2):
            nc.sync.dma_start(out=sw[:, :, :, r, :], in_=rsv[:, r, :, :, :])
        dn = dpool.tile([N, 768], FP, name="dn", tag="dn")
        nc.sync.dma_start(out=dn[:], in_=scr[:])
        dnv = dn[:].rearrange("p (r q) -> p r q", r=3)
        num = dpool.tile([N, BS], FP, name="num", tag="num")
        den = dpool.tile([N, BS], FP, name="den", tag="den")
        nc.vector.tensor_add(num[:], dnv[:, 1, :], dnv[:, 2, :])
        nc.vector.reciprocal_approx_fast(out=den[:], in_=dnv[:, 0, :])
        ov = dpool.tile([N, BS], FP, name="ov", tag="ov")
        nc.vector.tensor_mul(ov[:], num[:], den[:])
        nc.vector.tensor_scalar(
            out=ov[:], in0=ov[:], scalar1=sc["wvo"], scalar2=sc["bvo"] + sc["bo"],
            op0=ALU.mult, op1=ALU.add,
        )
        nc.sync.dma_start(out=out_t[:, b0 * BS:(b0 + 1) * BS], in_=ov[:])
        res_state["nflush"] = 0
        res_state["rs"] = None

    # --- main loop --------------------------------------------------------------
    pendings = []
    cur_qk = None
    vcur = [None]
    kcur = [None]
    NGRP = (PAIRS + GROUP - 1) // GROUP
    for g in range(NGRP):
        cls = group_class(g)
        p0 = g * GROUP
        npair = min(GROUP, PAIRS - p0)
        width = npair * 512
        members = []
        st = None
        sc16 = None
        for s in range(npair):
            p = p0 + s
            b, n = divmod(p, N)
            if n == 0:
                vcur[0], kcur[0] = load_vz(b)
            if p % SPD == 0:
                cur_qk = load_qz(p // SPD)
            sl = (p % SPD) * PW
            if cls == "C":
                if s == 0:
                    sc16 = spool.tile([128, GROUP * 512], F16, name="sc16", tag="sc16")
                qb = qpool.tile([128, 256], F16, name="qb", tag="qb")
                nc.gpsimd.partition_broadcast(qb[:], cur_qk[:][0:1, sl:sl + 256])
                for t in (0, 1):
                    nc.vector.tensor_scalar(
                        out=sc16[:, s * 512 + t * 256: s * 512 + (t + 1) * 256],
                        in0=qb[:], scalar1=kcur[0][:][:, t, n:n + 1],
                        scalar2=None, op0=ALU.mult, op1=ALU.bypass,
                    )
            else:
                if s == 0:
                    st = ps_stage.tile([128, GROUP * 512], FP, name="st", tag="st")
                nc.tensor.matmul(
                    st[:, s * 512:(s + 1) * 512],
                    lhsT=cur_qk[:][:, sl + 512: sl + PW],
                    rhs=cur_qk[:][:, sl: sl + 512],
                    start=True, stop=True,
                    tile_position=(0, 0),
                )
            members.append((s, b, n, vcur[0]))

        if cls == "B":
            a16 = spool.tile([128, GROUP * 512], I16, name="a16", tag="a16")
            b16 = spool.tile([128, GROUP * 512], I16, name="b16", tag="b16")
            nc.vector.tensor_scalar(
                out=a16[:, 0:width], in0=st[:][:, 0:width],
                scalar1=SCHR_SCALE, scalar2=SCHR_B1, op0=ALU.mult, op1=ALU.add,
            )
            nc.vector.tensor_scalar(
                out=b16[:, 0:width], in0=st[:][:, 0:width],
                scalar1=SCHR_SCALE, scalar2=SCHR_B2, op0=ALU.mult, op1=ALU.add,
            )
            rhs_tiles = [(a16[:].bitcast(F16), 0), (b16[:].bitcast(F16), 3)]
        else:
            e = epool.tile([128, GROUP * 512], F16, name="e", tag="e")
            src = sc16[:] if cls == "C" else st[:]
            nc.scalar.activation(e[:][:, 0:width], src[:, 0:width], AF.Exp)
            rhs_tiles = [(e[:], 0)]
        pendings.append((rhs_tiles, members))
        if len(pendings) > 2:
            emit_reduces(pendings.pop(0))
    for pend in pendings:
        emit_reduces(pend)
    assert res_state["count"] == 0 and res_state["nflush"] == 0, (
        "pair count must be a multiple of 32 (one block per division batch)"
    )


_CACHE: dict = {}


def _get_nc(sc, reps: int = 1) -> bass.Bass:
    key = (tuple(sorted(sc.items())), reps)
    if key not in _CACHE:
        _CACHE[key] = build_kernel_module(sc, reps)
    return _CACHE[key]


def make_in_maps(query, key, value, in_proj_w, in_proj_b, out_proj_w, out_proj_b):
    q = np.ascontiguousarray(np.asarray(query, dtype=np.float32).reshape(L, N))
    k = np.ascontiguousarray(np.asarray(key, dtype=np.float32).reshape(L, N))
    vv = np.ascontiguousarray(np.asarray(value, dtype=np.float32).reshape(L, N))
    wq, wk, wv = [float(x) for x in np.asarray(in_proj_w, dtype=np.float32).reshape(3)]
    bq, bk, bv = [float(x) for x in np.asarray(in_proj_b, dtype=np.float32).reshape(3)]
    wo = float(np.asarray(out_proj_w, dtype=np.float32).reshape(1)[0])
    bo = float(np.asarray(out_proj_b, dtype=np.float32).reshape(1)[0])
    sc = {"wvo": float(np.float32(wo) * np.float32(wv)),
          "bvo": float(np.float32(wo) * np.float32(bv)), "bo": bo}

    q16 = (q * np.float32(wq) + np.float32(bq)).astype(np.float16)
    k16 = (k * np.float32(wk) + np.float32(bk)).astype(np.float16)
    vhi = vv.astype(np.float16)
    vlo = (vv - vhi.astype(np.float32)).astype(np.float16)

    p = np.arange(PAIRS)
    b, n = p // N, p % N
    ar = np.arange(256)
    ar128 = np.arange(128)

    in_maps = []
    for c in range(NCORES):
        sl = slice(c * LS, (c + 1) * LS)
        qc = np.ascontiguousarray(q16[sl].T)   # [N, LS]
        kc = np.ascontiguousarray(k16[sl].T)
        qrows = qc[n[:, None], (b * BS)[:, None] + ar]            # [512, 256]
        qzc = np.zeros((PAIRS, 2, PW), np.float16)
        qzc[:, 0, 0:256] = qrows
        qzc[:, 1, 256:512] = qrows
        for t in (0, 1):
            qzc[:, t, 512:640] = kc[n[:, None], (b * BS + t * 128)[:, None] + ar128]
        # vz[b, p, (t, n, c6)]: cols 0:3 = (1, vhi, vlo), 3:6 = sqrt2 * same
        vzc = np.empty((BPC, 128, 2, N, 6), np.float16)
        vzc[:, :, :, :, 0] = 1.0
        vzc[:, :, :, :, 1] = vhi[sl].reshape(BPC, 2, 128, N).transpose(0, 2, 1, 3)
        vzc[:, :, :, :, 2] = vlo[sl].reshape(BPC, 2, 128, N).transpose(0, 2, 1, 3)
        vzc[:, :, :, :, 3:6] = (
            vzc[:, :, :, :, 0:3].astype(np.float32) * np.float32(SQRT2)
        ).astype(np.float16)
        # kz[b, p, (t, n)] = k'[b*256 + t*128 + p, n]
        kzc = np.ascontiguousarray(
            k16[sl].astype(np.float32).reshape(BPC, 2, 128, N).transpose(0, 2, 1, 3)
        )
        in_maps.append({
            "qz": np.ascontiguousarray(qzc),
            "vz": np.ascontiguousarray(vzc.reshape(BPC, 128, 2 * N * 6)),
            "kz": np.ascontiguousarray(kzc.reshape(BPC, 128, 2 * N)),
        })
    return in_maps, sc


def run(in_maps, sc, **kwargs):
    return run_bass_kernel_spmd(_get_nc(sc), in_maps, list(range(NCORES)), **kwargs)


def assemble(results) -> np.ndarray:
    outs = [np.asarray(results[c]["out_t"], dtype=np.float32).T for c in range(NCORES)]
    return np.ascontiguousarray(np.concatenate(outs, axis=0)).reshape(L, N, 1)


def kernel(query, key, value, in_proj_w, in_proj_b, out_proj_w, out_proj_b):
    in_maps, sc = make_in_maps(
        query, key, value, in_proj_w, in_proj_b, out_proj_w, out_proj_b
    )
    res = run(in_maps, sc)
    return assemble(res.results)


# revision 12
# speedup vs baseline: 1.1365x; 1.0314x over previous
"""Trainium2 Bass kernel for nn_BlockCrossAttn (block-diagonal attention, E=H=1).

Math per (block b, batch n) pair (256-long vectors q', k', v of the block):
    q' = wq*Q + bq ; k' = wk*K + bk
    soft[q,k] = softmax_k(q'[q] * k'[k])
    out[q] = wvo * (sum_k soft[q,k] * V[k]) + (bvo + bo)

Sharding: 128 blocks of 256 rows; 16 blocks per core across 8 cores.

Per-core pipeline (512 pairs, 171 groups of <=3).  Scores S^T[k, (t,q)] are
PSUM-drain-bound on the PE (~0.9ns/col regardless of matmul shape), and exp
is ScalarE-bound (~0.96ns/col), so groups are split across three classes to
balance all four engines:

  - class A (default): one K=2 PE matmul per pair (host-packed zero-padded
    [2, 640] = [q|0|k0 ; 0|q|k1] fp16 operands) -> PSUM bank; ScalarE exp
    [128, 1536] -> E fp16 in SBUF.
  - class C (PE relief, g%8 in {1,3,5}): GpSimd partition-broadcasts the q
    row; VectorE tensor_scalar (per-partition k scalar AP) builds the score
    span in SBUF at 2-byte rate; ScalarE exp from SBUF.
  - class B (ScalarE relief, g%8==7): PE scores as in A, then VectorE
    computes a two-term Schraudolph exp E = f16^(i16(s*1024*log2e + B1)) +
    sqrt2 * f16^(i16(s*1024*log2e + B2)) (~0.8% rel err, common scale
    cancels in softmax).  The sqrt2 combine is folded into the reduction:
    the second term uses a sqrt2-scaled [ones,v_hi,v_lo] triplet and
    accumulates into the same PSUM region (4 matmuls instead of 2).

  - Reduction: lhsT = [ones, v_hi, v_lo] fp16 (host-packed), rhs = E; the
    k-half matmuls accumulate into one PSUM [3, 256] region, 8 pairs per
    result bank.  VectorE flushes banks; a DRAM scratch bounce re-lays 32
    pairs into [32, 768]; num = hi+lo, reciprocal_approx_fast, multiply,
    affine epilogue; one contiguous DMA per block to the n-major output.
"""

from contextlib import ExitStack

import numpy as np

import concourse.bacc as bacc
import concourse.bass as bass
import concourse.tile as tile
from concourse import mybir
from concourse.bass_utils import run_bass_kernel_spmd

FP = mybir.dt.float32
F16 = mybir.dt.float16
I16 = mybir.dt.int16
AF = mybir.ActivationFunctionType
ALU = mybir.AluOpType

L = 32768          # sequence length
N = 32             # batch
BS = 256           # block size
NB = L // BS       # 128 blocks
NCORES = 8
BPC = NB // NCORES  # 16 blocks per core
LS = BPC * BS       # 4096 rows per core shard

GROUP = 3           # pairs per PSUM stage group (3 banks)
PAIRS = BPC * N     # 512 pairs per core
PW = 640            # qz cols per pair: rhs 512 | lhsT 128
SPD = 16            # pairs per staging DMA

# Schraudolph constants (fp16-domain, int16-bitcast, +0.25 hedges the
# rint-vs-trunc convert ambiguity); E = a + sqrt2*b via the dual triplet.
SCHR_SCALE = float(1024.0 * np.log2(np.e))
SCHR_B1 = 15305.25
SCHR_B2 = 14792.25
SQRT2 = float(np.sqrt(2.0))


def group_class(g):
    if g % 2 == 1:
        return "C"
    if g % 16 == 8:
        return "B"
    return "A"


def build_kernel_module(sc, reps: int = 1) -> bass.Bass:
    nc = bacc.Bacc("TRN2", target_bir_lowering=False, debug=False, num_devices=NCORES)
    qz = nc.declare_dram_parameter("qz", [PAIRS, 2, PW], F16, isOutput=False)
    vz = nc.declare_dram_parameter("vz", [BPC, 128, 2 * N * 6], F16, isOutput=False)
    kz = nc.declare_dram_parameter("kz", [BPC, 128, 2 * N], FP, isOutput=False)
    out_t = nc.declare_dram_parameter("out_t", [N, LS], FP, isOutput=True)

    with tile.TileContext(nc) as tc:
        with ExitStack() as ctx:
            if reps == 1:
                _emit(ctx, tc, qz, vz, kz, out_t, sc)
            else:
                with tc.For_i(0, reps, 1):
                    _emit(ctx, tc, qz, vz, kz, out_t, sc)
    nc.compile()
    return nc


def _emit(ctx, tc, qz, vz, kz, out_t, sc):
    nc = tc.nc

    stage = ctx.enter_context(tc.tile_pool(name="stage", bufs=2))
    vpool = ctx.enter_context(tc.tile_pool(name="vpool", bufs=2))
    kpool = ctx.enter_context(tc.tile_pool(name="kpool", bufs=2))
    qpool = ctx.enter_context(tc.tile_pool(name="qpool", bufs=8))
    epool = ctx.enter_context(tc.tile_pool(name="epool", bufs=4))
    spool = ctx.enter_context(tc.tile_pool(name="spool", bufs=6))
    dpool = ctx.enter_context(tc.tile_pool(name="dpool", bufs=2))
    ps_stage = ctx.enter_context(tc.tile_pool(name="ps_stage", bufs=2, space="PSUM"))
    ps_res = ctx.enter_context(tc.tile_pool(name="ps_res", bufs=2, space="PSUM"))
    drs = ctx.enter_context(tc.tile_pool(name="drs", bufs=2, space="DRAM"))

    def load_qz(c):
        qk = stage.tile([2, SPD * PW], F16, name="qk", tag="qk")
        nc.sync.dma_start(
            out=qk[:].rearrange("p (s w) -> p s w", s=SPD),
            in_=qz[c * SPD:(c + 1) * SPD].rearrange("s p w -> p s w"),
        )
        return qk

    def load_vz(b):
        vc = vpool.tile([128, 2, N, 6], F16, name="vc", tag="vc")
        nc.sync.dma_start(out=vc[:].rearrange("p t n c -> p (t n c)"), in_=vz[b])
        kc = kpool.tile([128, 2, N], FP, name="kc", tag="kc")
        nc.sync.dma_start(out=kc[:].rearrange("p t n -> p (t n)"), in_=kz[b])
        return vc, kc

    # --- reduction / division ---------------------------------------------------
    res_state = {"tile": None, "count": 0, "nflush": 0, "rs": None, "first_g": 0}

    def emit_reduces(pend):
        rhs_tiles, members = pend
        for (s, b, n, vc) in members:
            g = b * N + n
            p8 = res_state["count"]
            if p8 == 0:
                res_state["tile"] = ps_res.tile([128, 512], FP, name="res", tag="res")
                if res_state["nflush"] == 0:
                    res_state["rs"] = dpool.tile([128, 2048], FP, name="rs", tag="rs")
                    res_state["first_g"] = g
            j, h = p8 % 4, p8 // 4
            nmm = len(rhs_tiles) * 2
            i = 0
            for (e, c0) in rhs_tiles:
                for t in (0, 1):
                    nc.tensor.matmul(
                        res_state["tile"][32 * j:32 * j + 3, h * 256:(h + 1) * 256],
                        lhsT=vc[:][:, t, n, c0:c0 + 3],
                        rhs=e[:][:, s * 512 + t * 256: s * 512 + (t + 1) * 256],
                        start=(i == 0), stop=(i == nmm - 1),
                        tile_position=(0, 32 * j),
                    )
                    i += 1
            res_state["count"] += 1
            if res_state["count"] == 8:
                m = res_state["nflush"]
                nc.vector.tensor_copy(
                    res_state["rs"][:, m * 512:(m + 1) * 512], res_state["tile"][:]
                )
                res_state["count"] = 0
                res_state["tile"] = None
                res_state["nflush"] += 1
                if res_state["nflush"] == 4:
                    division_batch()

    def division_batch():
        b0 = res_state["first_g"] // N
        rs = res_state["rs"]
        scr = drs.tile([N, 768], FP, name="scr", tag="scr")
        rsv = rs[:].rearrange("(j p2) (m h q) -> j p2 m h q", j=4, m=4, h=2)
        sw = scr[:].rearrange("(m h j) (r q) -> j m h r q", m=4, h=2, r=3)
        for r in (0, 1, 2):
            nc.sync.dma_start(out=sw[:, :, :, r, :], in_=rsv[:, r, :, :, :])
        dn = dpool.tile([N, 768], FP, name="dn", tag="dn")
        nc.sync.dma_start(out=dn[:], in_=scr[:])
        dnv = dn[:].rearrange("p (r q) -> p r q", r=3)
        num = dpool.tile([N, BS], FP, name="num", tag="num")
        den = dpool.tile([N, BS], FP, name="den", tag="den")
        nc.vector.tensor_add(num[:], dnv[:, 1, :], dnv[:, 2, :])
        nc.vector.reciprocal_approx_fast(out=den[:], in_=dnv[:, 0, :])
        ov = dpool.tile([N, BS], FP, name="ov", tag="ov")
        nc.vector.tensor_mul(ov[:], num[:], den[:])
        nc.vector.tensor_scalar(
            out=ov[:], in0=ov[:], scalar1=sc["wvo"], scalar2=sc["bvo"] + sc["bo"],
            op0=ALU.mult, op1=ALU.add,
        )
        nc.sync.dma_start(out=out_t[:, b0 * BS:(b0 + 1) * BS], in_=ov[:])
        res_state["nflush"] = 0
        res_state["rs"] = None

    # --- main loop --------------------------------------------------------------
    pendings = []
    cur_qk = None
    vcur = [None]
    kcur = [None]
    NGRP = (PAIRS + GROUP - 1) // GROUP
    for g in range(NGRP):
        cls = group_class(g)
        p0 = g * GROUP
        npair = min(GROUP, PAIRS - p0)
        width = npair * 512
        members = []
        st = None
        sc16 = None
        for s in range(npair):
            p = p0 + s
            b, n = divmod(p, N)
            if n == 0:
                vcur[0], kcur[0] = load_vz(b)
            if p % SPD == 0:
                cur_qk = load_qz(p // SPD)
            sl = (p % SPD) * PW
            if cls == "C":
                if s == 0:
                    sc16 = spool.tile([128, GROUP * 512], F16, name="sc16", tag="sc16")
                qb = qpool.tile([128, 256], F16, name="qb", tag="qb")
                nc.gpsimd.partition_broadcast(qb[:], cur_qk[:][0:1, sl:sl + 256])
                for t in (0, 1):
                    nc.vector.tensor_scalar(
                        out=sc16[:, s * 512 + t * 256: s * 512 + (t + 1) * 256],
                        in0=qb[:], scalar1=kcur[0][:][:, t, n:n + 1],
                        scalar2=None, op0=ALU.mult, op1=ALU.bypass,
                    )
            else:
                if s == 0:
                    st = ps_stage.tile([128, GROUP * 512], FP, name="st", tag="st")
                nc.tensor.matmul(
                    st[:, s * 512:(s + 1) * 512],
                    lhsT=cur_qk[:][:, sl + 512: sl + PW],
                    rhs=cur_qk[:][:, sl: sl + 512],
                    start=True, stop=True,
                    tile_position=(0, 0),
                )
            members.append((s, b, n, vcur[0]))

        if cls == "B":
            a16 = spool.tile([128, GROUP * 512], I16, name="a16", tag="a16")
            b16 = spool.tile([128, GROUP * 512], I16, name="b16", tag="b16")
            nc.vector.tensor_scalar(
                out=a16[:, 0:width], in0=st[:][:, 0:width],
                scalar1=SCHR_SCALE, scalar2=SCHR_B1, op0=ALU.mult, op1=ALU.add,
            )
            nc.vector.tensor_scalar(
                out=b16[:, 0:width], in0=st[:][:, 0:width],
                scalar1=SCHR_SCALE, scalar2=SCHR_B2, op0=ALU.mult, op1=ALU.add,
            )
            rhs_tiles = [(a16[:].bitcast(F16), 0), (b16[:].bitcast(F16), 3)]
        else:
            e = epool.tile([128, GROUP * 512], F16, name="e", tag="e")
            src = sc16[:] if cls == "C" else st[:]
            nc.scalar.activation(e[:][:, 0:width], src[:, 0:width], AF.Exp)
            rhs_tiles = [(e[:], 0)]
        pendings.append((rhs_tiles, members))
        if len(pendings) > 2:
            emit_reduces(pendings.pop(0))
    for pend in pendings:
        emit_reduces(pend)
    assert res_state["count"] == 0 and res_state["nflush"] == 0, (
        "pair count must be a multiple of 32 (one block per division batch)"
    )


_CACHE: dict = {}


def _get_nc(sc, reps: int = 1) -> bass.Bass:
    key = (tuple(sorted(sc.items())), reps)
    if key not in _CACHE:
        _CACHE[key] = build_kernel_module(sc, reps)
    return _CACHE[key]


def make_in_maps(query, key, value, in_proj_w, in_proj_b, out_proj_w, out_proj_b):
    q = np.ascontiguousarray(np.asarray(query, dtype=np.float32).reshape(L, N))
    k = np.ascontiguousarray(np.asarray(key, dtype=np.float32).reshape(L, N))
    vv = np.ascontiguousarray(np.asarray(value, dtype=np.float32).reshape(L, N))
    wq, wk, wv = [float(x) for x in np.asarray(in_proj_w, dtype=np.float32).reshape(3)]
    bq, bk, bv = [float(x) for x in np.asarray(in_proj_b, dtype=np.float32).reshape(3)]
    wo = float(np.asarray(out_proj_w, dtype=np.float32).reshape(1)[0])
    bo = float(np.asarray(out_proj_b, dtype=np.float32).reshape(1)[0])
    sc = {"wvo": float(np.float32(wo) * np.float32(wv)),
          "bvo": float(np.float32(wo) * np.float32(bv)), "bo": bo}

    q16 = (q * np.float32(wq) + np.float32(bq)).astype(np.float16)
    k16 = (k * np.float32(wk) + np.float32(bk)).astype(np.float16)
    vhi = vv.astype(np.float16)
    vlo = (vv - vhi.astype(np.float32)).astype(np.float16)

    p = np.arange(PAIRS)
    b, n = p // N, p % N
    ar = np.arange(256)
    ar128 = np.arange(128)

    in_maps = []
    for c in range(NCORES):
        sl = slice(c * LS, (c + 1) * LS)
        qc = np.ascontiguousarray(q16[sl].T)   # [N, LS]
        kc = np.ascontiguousarray(k16[sl].T)
        qrows = qc[n[:, None], (b * BS)[:, None] + ar]            # [512, 256]
        qzc = np.zeros((PAIRS, 2, PW), np.float16)
        qzc[:, 0, 0:256] = qrows
        qzc[:, 1, 256:512] = qrows
        for t in (0, 1):
            qzc[:, t, 512:640] = kc[n[:, None], (b * BS + t * 128)[:, None] + ar128]
        # vz[b, p, (t, n, c6)]: cols 0:3 = (1, vhi, vlo), 3:6 = sqrt2 * same
        vzc = np.empty((BPC, 128, 2, N, 6), np.float16)
        vzc[:, :, :, :, 0] = 1.0
        vzc[:, :, :, :, 1] = vhi[sl].reshape(BPC, 2, 128, N).transpose(0, 2, 1, 3)
        vzc[:, :, :, :, 2] = vlo[sl].reshape(BPC, 2, 128, N).transpose(0, 2, 1, 3)
        vzc[:, :, :, :, 3:6] = (
            vzc[:, :, :, :, 0:3].astype(np.float32) * np.float32(SQRT2)
        ).astype(np.float16)
        # kz[b, p, (t, n)] = k'[b*256 + t*128 + p, n]
        kzc = np.ascontiguousarray(
            k16[sl].astype(np.float32).reshape(BPC, 2, 128, N).transpose(0, 2, 1, 3)
        )
        in_maps.append({
            "qz": np.ascontiguousarray(qzc),
            "vz": np.ascontiguousarray(vzc.reshape(BPC, 128, 2 * N * 6)),
            "kz": np.ascontiguousarray(kzc.reshape(BPC, 128, 2 * N)),
        })
    return in_maps, sc


def run(in_maps, sc, **kwargs):
    return run_bass_kernel_spmd(_get_nc(sc), in_maps, list(range(NCORES)), **kwargs)


def assemble(results) -> np.ndarray:
    outs = [np.asarray(results[c]["out_t"], dtype=np.float32).T for c in range(NCORES)]
    return np.ascontiguousarray(np.concatenate(outs, axis=0)).reshape(L, N, 1)


def kernel(query, key, value, in_proj_w, in_proj_b, out_proj_w, out_proj_b):
    in_maps, sc = make_in_maps(
        query, key, value, in_proj_w, in_proj_b, out_proj_w, out_proj_b
    )
    res = run(in_maps, sc)
    return assemble(res.results)
